# revision 1
# baseline (speedup 1.0000x reference)
"""DrugTargetGNN on 8 Trainium2 NeuronCores (Bass/Tile, SPMD).

Destination-sharded GCN message passing; see task notes. One instruction
stream for all 8 cores: per-tile chunk counts are the max over cores,
pad edges carry col_local=-1 (zero row in the one-hot S matrix).

Per 128-dest tile:
  indirect-gather x[row_e] (one row/partition/chunk, 4 SWDGE queues)
  -> DVE: msgs = gathered * norm (fused over the tile's chunks)
  -> DVE: S[e,d] = is_equal(col_local[e], iota[d]) (fused)
  -> PE:  agg_T[F_in, 128d] += msgs.T @ S  (PSUM, chunks accumulate)
  -> PE:  x_next[128d, F_out] = agg_T.T @ W (PSUM, K-chunk accumulate)
  -> ACT: relu -> grouped HWDGE writes of the node shard.
AllGather full x_next after layers 1,2. Pool = same S-matmul over
contiguous node chunks into a 128-graph window; windows AllGather'd and
merged positionally on all cores; MLP heads replicated. All fp32.
"""
import math
import os
import numpy as np

import concourse.bass as bass
import concourse.tile as tile
from concourse import bacc, mybir
from concourse.bass_utils import run_bass_kernel_spmd

F32 = mybir.dt.float32
I32 = mybir.dt.int32
AF = mybir.ActivationFunctionType
OP = mybir.AluOpType

NCORE = 8
G = 512
ND, ED, NP_, EP = 100000, 500000, 200000, 2000000
SHARD_D, SHARD_P = ND // NCORE, NP_ // NCORE
P = 128

_CACHE = {}


# ----------------------------------------------------------------- host prep
def _prep_tower(edge_index, edge_attr, N, shard):
    row = np.asarray(edge_index[0], np.int64)
    col = np.asarray(edge_index[1], np.int64)
    w = np.asarray(edge_attr[:, 0], np.float32)
    deg = (np.bincount(col, minlength=N) + 1).astype(np.float32)
    dinv = (deg ** np.float32(-0.5)).astype(np.float32)
    norm = (dinv[row] * w * dinv[col]).astype(np.float32)

    ar = np.arange(N, dtype=np.int64)
    row = np.concatenate([row, ar])
    colf = np.concatenate([col, ar])
    norm = np.concatenate([norm, (dinv * dinv).astype(np.float32)])

    order = np.argsort(colf, kind="stable")
    row, colf, norm = row[order], colf[order], norm[order]

    core = colf // shard
    local = colf - core * shard
    tl = local // P
    col_local = (local % P).astype(np.float32)
    ntiles = math.ceil(shard / P)

    bucket = (core * ntiles + tl).astype(np.int64)
    cnt = np.bincount(bucket, minlength=NCORE * ntiles).reshape(NCORE, ntiles)
    nchunks = np.maximum(1, -(-cnt.max(axis=0) // P)).astype(np.int64)
    chunk_off = np.concatenate([[0], np.cumsum(nchunks)])
    C = int(chunk_off[-1])

    starts = np.concatenate([[0], np.cumsum(cnt.ravel())])[:-1]
    rank = np.arange(len(colf)) - starts[bucket]

    gidx = np.zeros((NCORE, P, C), np.int32)
    nrm = np.zeros((NCORE, P, C), np.float32)
    cl = np.full((NCORE, P, C), -1.0, np.float32)
    cc = (chunk_off[tl] + rank // P).astype(np.int64)
    pp = (rank % P).astype(np.int64)
    gidx[core, pp, cc] = row.astype(np.int32)
    nrm[core, pp, cc] = norm
    cl[core, pp, cc] = col_local

    tile_rows = [min(P, shard - t * P) for t in range(ntiles)]
    return dict(gidx=gidx, nrm=nrm, cl=cl, nchunks=[int(x) for x in nchunks],
                C=C, ntiles=ntiles, tile_rows=tile_rows)


def _prep_pool(batch, N, shard):
    batch = np.asarray(batch, np.int64)
    ntiles = math.ceil(shard / P)
    n = np.arange(N, dtype=np.int64)
    core = n // shard
    local = n - core * shard
    g = batch - 64 * core + 32
    assert g.min() >= 0 and g.max() < P, "graph window overflow"
    poolcol = np.full((NCORE, P, ntiles), -1.0, np.float32)
    poolcol[core, local % P, local // P] = g.astype(np.float32)
    cnt = np.bincount(batch, minlength=G).astype(np.float32)
    inv = (1.0 / np.maximum(cnt, 1.0)).astype(np.float32)
    return poolcol, np.tile(inv[None, :], (P, 1))


def _fsplit(F):
    out, o = [], 0
    while o < F:
        c = min(P, F - o)
        out.append((o, c))
        o += c
    return out


# ------------------------------------------------------------- program build
NQ = int(os.environ.get("K_NQ", "4"))
VARIANT = os.environ.get("K_VARIANT", "full")


def _build(meta):
    nc = bacc.Bacc("TRN2", target_bir_lowering=False, debug=False,
                   num_devices=NCORE, num_swdge_queues=NQ)
    qnames = ["qPoolDynamic", "qPoolDynamic1", "qPoolDynamic2",
              "qPoolDynamic3"][:NQ]
    qi = [0]

    def gather(out_ap, src_ap, off_ap):
        inst = nc.gpsimd.indirect_dma_start(
            out=out_ap, out_offset=None, in_=src_ap,
            in_offset=bass.IndirectOffsetOnAxis(ap=off_ap, axis=0))
        if NQ > 1:
            inst.ins.queue = qnames[qi[0] % NQ]
        qi[0] += 1

    def ein(name, shape, dt=F32):
        return nc.dram_tensor(name, shape, dt, kind="ExternalInput")

    drug_x = ein("drug_x", [ND, 78])
    protein_x = ein("protein_x", [NP_, 20])
    d_gidx = ein("d_gidx", [P, meta["d"]["C"]], I32)
    d_nrm = ein("d_nrm", [P, meta["d"]["C"]])
    d_cl = ein("d_cl", [P, meta["d"]["C"]])
    p_gidx = ein("p_gidx", [P, meta["p"]["C"]], I32)
    p_nrm = ein("p_nrm", [P, meta["p"]["C"]])
    p_cl = ein("p_cl", [P, meta["p"]["C"]])
    d_pc = ein("d_pc", [P, meta["d"]["ntiles"]])
    p_pc = ein("p_pc", [P, meta["p"]["ntiles"]])
    iota_c = ein("iota_c", [P, P])
    d_inv = ein("d_inv", [P, G])
    p_inv = ein("p_inv", [P, G])

    wshapes = {"dW1": (78, 78), "dW2": (78, 156), "dW3": (156, 312),
               "pW1": (20, 20), "pW2": (20, 40), "pW3": (40, 80),
               "dL1_w": (312, 1024), "dL2_w": (1024, 64),
               "pL1_w": (80, 1024), "pL2_w": (1024, 64),
               "fW1": (128, 1024), "fW2": (1024, 512), "fW3": (512, 1)}
    wt = {n: ein(n, list(s)) for n, s in wshapes.items()}
    bt = {n: ein(n, list(s)) for n, s in
          [("dL1_b", (P, 8)), ("dL2_b", (64, 1)), ("pL1_b", (P, 8)),
           ("pL2_b", (64, 1)), ("fb1", (P, 8)), ("fb2", (P, 4)),
           ("fb3", (1, 1))]}

    out_t = nc.dram_tensor("out", [1, G], F32, kind="ExternalOutput")
    dbg = {}
    if os.environ.get("K_DEBUG"):
        dbg["d_loc1"] = nc.dram_tensor("dbg_d_loc1", [SHARD_D, 78], F32,
                                       kind="ExternalOutput")
        dbg["d_x3"] = nc.dram_tensor("dbg_d_x3", [SHARD_D, 312], F32,
                                     kind="ExternalOutput")
        dbg["p_x3"] = nc.dram_tensor("dbg_p_x3", [SHARD_P, 80], F32,
                                     kind="ExternalOutput")
        dbg["d_win"] = nc.dram_tensor("dbg_d_win", [312, P], F32,
                                      kind="ExternalOutput")
        dbg["p_win"] = nc.dram_tensor("dbg_p_win", [80, P], F32,
                                      kind="ExternalOutput")
        dbg["c_sb"] = nc.dram_tensor("dbg_c_sb", [P, G], F32,
                                     kind="ExternalOutput")

    def din(name, shape):
        return nc.dram_tensor(name, shape, F32, kind="Internal")

    def dsh(name, shape):
        return nc.dram_tensor(name, shape, F32, kind="Internal",
                              addr_space="Shared")

    d_loc1, d_full1 = din("d_loc1", [SHARD_D, 78]), din("d_full1", [ND, 78])
    d_loc2, d_full2 = din("d_loc2", [SHARD_D, 156]), din("d_full2", [ND, 156])
    d_x3 = din("d_x3", [SHARD_D, 312])
    p_loc1, p_full1 = din("p_loc1", [SHARD_P, 20]), din("p_full1", [NP_, 20])
    p_loc2, p_full2 = din("p_loc2", [SHARD_P, 40]), din("p_full2", [NP_, 40])
    p_x3 = din("p_x3", [SHARD_P, 80])
    d_pool_in, d_pool_out = din("d_pool_in", [312 * P]), din("d_pool_out", [NCORE * 312 * P])
    p_pool_in, p_pool_out = din("p_pool_in", [80 * P]), din("p_pool_out", [NCORE * 80 * P])

    rg = [list(range(NCORE))]

    from contextlib import ExitStack
    with tile.TileContext(nc) as tc, ExitStack() as stack:
        cpool = stack.enter_context(tc.tile_pool(name="const", bufs=1))

        def load_const(src, shape, dt=F32, tag=None):
            t = cpool.tile(shape, dt, tag=tag or src.name)
            nc.sync.dma_start(t[:], src[:])
            return t

        iota_t = load_const(iota_c, [P, P])
        meta_t = {}
        for nme, src in [("d_gidx", d_gidx), ("d_nrm", d_nrm), ("d_cl", d_cl),
                         ("p_gidx", p_gidx), ("p_nrm", p_nrm), ("p_cl", p_cl),
                         ("d_pc", d_pc), ("p_pc", p_pc),
                         ("d_inv", d_inv), ("p_inv", p_inv)]:
            meta_t[nme] = load_const(src, list(src.shape),
                                     I32 if "gidx" in nme else F32)

        def load_w(names):
            d = {}
            for nme in names:
                K, Nn = wshapes[nme]
                d[nme] = []
                for o, csz in _fsplit(K):
                    t = cpool.tile([csz, Nn], F32, tag=f"{nme}_{o}")
                    nc.sync.dma_start(t[:], wt[nme][o:o + csz, :])
                    d[nme].append(t)
            return d

        wsb = load_w(["dW1", "dW2", "dW3", "pW1", "pW2", "pW3"])

        def m_of(tw):
            return meta[tw]

        def allgather(loc, full):
            nc.gpsimd.collective_compute(
                "AllGather", OP.bypass, replica_groups=rg,
                ins=[loc[:]], outs=[full[:]])

        # ------------------------------- phase 1: GCN towers + pooling + AGs
        with (
            tc.tile_pool(name="gath", bufs=6) as gpool,
            tc.tile_pool(name="work", bufs=3) as wpool,
            tc.tile_pool(name="outw", bufs=2) as opool,
            tc.tile_pool(name="apsum", bufs=2, space="PSUM") as apsum,
            tc.tile_pool(name="xpsum", bufs=2, space="PSUM") as xpsum,
            tc.tile_pool(name="ppsum", bufs=1, space="PSUM") as ppsum,
        ):
            def gcn_layer(tw, src_ap, dst, F_in, F_out, wtiles, ngrp=8):
                m = m_of(tw)
                gx = meta_t[f"{tw}_gidx"]
                nr = meta_t[f"{tw}_nrm"]
                cl = meta_t[f"{tw}_cl"]
                fch = _fsplit(F_in)
                cptr = 0
                grp, grp_start, go = [], 0, None
                for t, ncnk in enumerate(m["nchunks"]):
                    rows = m["tile_rows"][t]
                    gat = gpool.tile([P, ncnk * F_in], F32, tag="gat")
                    for c in range(ncnk):
                        gather(gat[:, c * F_in:(c + 1) * F_in], src_ap,
                               gx[:, cptr + c:cptr + c + 1])
                    msgs = gpool.tile([P, ncnk * F_in], F32, tag="msgs")
                    nc.vector.tensor_tensor(
                        out=msgs[:].rearrange("p (c f) -> p c f", c=ncnk),
                        in0=gat[:].rearrange("p (c f) -> p c f", c=ncnk),
                        in1=nr[:, cptr:cptr + ncnk, None].to_broadcast(
                            [P, ncnk, F_in]),
                        op=OP.mult)
                    s_t = gpool.tile([P, ncnk * P], F32, tag="s")
                    nc.vector.tensor_tensor(
                        out=s_t[:].rearrange("p (c d) -> p c d", c=ncnk),
                        in0=cl[:, cptr:cptr + ncnk, None].to_broadcast(
                            [P, ncnk, P]),
                        in1=iota_t[:, None, :].to_broadcast([P, ncnk, P]),
                        op=OP.is_equal)
                    agg = apsum.tile([P, len(fch) * P], F32, tag="agg")
                    for fi, (fo, fw) in enumerate(fch):
                        for c in range(ncnk):
                            nc.tensor.matmul(
                                out=agg[:fw, fi * P:(fi + 1) * P],
                                lhsT=msgs[:, c * F_in + fo:c * F_in + fo + fw],
                                rhs=s_t[:, c * P:(c + 1) * P],
                                start=(c == 0), stop=(c == ncnk - 1))
                    asb = wpool.tile([P, len(fch) * P], F32, tag="aggsb")
                    for fi, (fo, fw) in enumerate(fch):
                        nc.scalar.activation(
                            asb[:fw, fi * P:(fi + 1) * P],
                            agg[:fw, fi * P:(fi + 1) * P], AF.Copy)
                    xn = xpsum.tile([P, F_out], F32, tag="xn")
                    for fi, (fo, fw) in enumerate(fch):
                        nc.tensor.matmul(
                            out=xn[:], lhsT=asb[:fw, fi * P:(fi + 1) * P],
                            rhs=wtiles[fi][:],
                            start=(fi == 0), stop=(fi == len(fch) - 1))
                    if not grp:
                        go = opool.tile([P, ngrp * F_out], F32, tag="go")
                        grp_start = t * P
                    nc.scalar.activation(
                        go[:rows, len(grp) * F_out:(len(grp) + 1) * F_out],
                        xn[:rows, :], AF.Relu)
                    grp.append(rows)
                    if len(grp) == ngrp or t == len(m["nchunks"]) - 1:
                        full = len(grp) if grp[-1] == P else len(grp) - 1
                        if full:
                            nc.sync.dma_start(
                                dst[grp_start:grp_start + full * P, :]
                                .rearrange("(c p) f -> p c f", p=P),
                                go[:, :full * F_out]
                                .rearrange("p (c f) -> p c f", c=full))
                        if grp[-1] != P:
                            nc.sync.dma_start(
                                dst[grp_start + full * P:
                                    grp_start + full * P + grp[-1], :],
                                go[:grp[-1],
                                   full * F_out:(full + 1) * F_out])
                        grp = []
                    cptr += ncnk
                assert cptr == m["C"]

            def pool_pass(tw, x3, F, pool_in):
                m = m_of(tw)
                pc = meta_t[f"{tw}_pc"]
                fch = _fsplit(F)
                pps = [ppsum.tile([fw, P], F32, tag=f"pp{fi}",
                                  name=f"pp{fi}")
                       for fi, (fo, fw) in enumerate(fch)]
                ntl = m["ntiles"]
                nb = 4
                first = True
                for b0 in range(0, ntl, nb):
                    bn = min(nb, ntl - b0)
                    last_rows = m["tile_rows"][b0 + bn - 1]
                    nfull = bn if last_rows == P else bn - 1
                    xt = wpool.tile([P, nb * F], F32, tag="poolx")
                    if last_rows != P:
                        nc.vector.memset(
                            xt[:, (bn - 1) * F:bn * F], 0.0)
                        nc.sync.dma_start(
                            xt[:last_rows, (bn - 1) * F:bn * F],
                            x3[(b0 + bn - 1) * P:
                               (b0 + bn - 1) * P + last_rows, :])
                    if nfull:
                        nc.sync.dma_start(
                            xt[:, :nfull * F]
                            .rearrange("p (c f) -> p c f", c=nfull),
                            x3[b0 * P:(b0 + nfull) * P, :]
                            .rearrange("(c p) f -> p c f", p=P))
                    st = wpool.tile([P, nb * P], F32, tag="pools")
                    nc.vector.tensor_tensor(
                        out=st[:, :bn * P].rearrange("p (c d) -> p c d", c=bn),
                        in0=pc[:, b0:b0 + bn, None].to_broadcast([P, bn, P]),
                        in1=iota_t[:, None, :].to_broadcast([P, bn, P]),
                        op=OP.is_equal)
                    for j in range(bn):
                        for fi, (fo, fw) in enumerate(fch):
                            nc.tensor.matmul(
                                out=pps[fi][:],
                                lhsT=xt[:, j * F + fo:j * F + fo + fw],
                                rhs=st[:, j * P:(j + 1) * P],
                                start=first,
                                stop=(b0 + bn >= ntl and j == bn - 1))
                        first = False
                pool2d = pool_in[:].rearrange("(f g) -> f g", g=P)
                for fi, (fo, fw) in enumerate(fch):
                    psb = wpool.tile([fw, P], F32, tag="poolsb")
                    nc.scalar.activation(psb[:], pps[fi][:],
                                         AF.Copy)
                    nc.sync.dma_start(pool2d[fo:fo + fw, :], psb[:])

            lvl = {"l1": 1, "l1ag": 2, "towers": 3, "full": 5}[VARIANT]
            gcn_layer("d", drug_x[:], d_loc1, 78, 78, wsb["dW1"])
            if lvl >= 2:
                allgather(d_loc1, d_full1)
            if lvl >= 3:
                gcn_layer("p", protein_x[:], p_loc1, 20, 20, wsb["pW1"])
                allgather(p_loc1, p_full1)
                gcn_layer("d", d_full1[:], d_loc2, 78, 156, wsb["dW2"])
                allgather(d_loc2, d_full2)
                gcn_layer("p", p_full1[:], p_loc2, 20, 40, wsb["pW2"])
                allgather(p_loc2, p_full2)
                gcn_layer("d", d_full2[:], d_x3, 156, 312, wsb["dW3"])
                gcn_layer("p", p_full2[:], p_x3, 40, 80, wsb["pW3"])
            if lvl >= 5:
                pool_pass("d", d_x3, 312, d_pool_in)
                allgather(d_pool_in, d_pool_out)
                pool_pass("p", p_x3, 80, p_pool_in)
                allgather(p_pool_in, p_pool_out)

        # --------------------------------- phase 2: merge pools + MLP heads
        if VARIANT != "full":
            with tc.tile_pool(name="stub", bufs=1) as spool:
                ob = spool.tile([1, G], F32, tag="ob")
                nc.vector.memset(ob[:], 0.0)
                nc.sync.dma_start(out_t[:], ob[:])
        elif True:
            with (
                tc.tile_pool(name="hconst", bufs=1) as hcpool,
                tc.tile_pool(name="hwork", bufs=2) as hpool,
                tc.tile_pool(name="hpers", bufs=1) as hkeep,
                tc.tile_pool(name="hpsum", bufs=2, space="PSUM") as hpsum,
            ):
                hw = {}
                for nme in ["dL1_w", "dL2_w", "pL1_w", "pL2_w", "fW1", "fW2",
                            "fW3"]:
                    K, Nn = wshapes[nme]
                    hw[nme] = []
                    for o, csz in _fsplit(K):
                        t = hcpool.tile([csz, Nn], F32, tag=f"{nme}_{o}")
                        nc.sync.dma_start(t[:], wt[nme][o:o + csz, :])
                        hw[nme].append(t)
                bsb = {}
                for nme, b in bt.items():
                    t = hcpool.tile(list(b.shape), F32, tag=nme)
                    nc.sync.dma_start(t[:], b[:])
                    bsb[nme] = t

                def pool_merge(tw, pool_out, F, inv_t):
                    fch = _fsplit(F)
                    accs = []
                    po = pool_out[:].rearrange("(k f g) -> k f g", k=NCORE, g=P)
                    for fi, (fo, fw) in enumerate(fch):
                        acc = hkeep.tile([fw, G], F32, tag=f"pacc{tw}{fi}")
                        nc.vector.memset(acc[:], 0.0)
                        for k in range(NCORE):
                            win = hpool.tile([fw, P], F32, tag="pwin")
                            nc.sync.dma_start(win[:], po[k, fo:fo + fw, :])
                            lo = 64 * k - 32
                            a, b = max(0, lo), min(G, lo + P)
                            nc.vector.tensor_tensor(
                                out=acc[:, a:b], in0=acc[:, a:b],
                                in1=win[:, a - lo:b - lo], op=OP.add)
                        nc.vector.tensor_tensor(out=acc[:], in0=acc[:],
                                                in1=inv_t[:fw, :], op=OP.mult)
                        accs.append(acc)
                    return accs

                d_acc = pool_merge("d", d_pool_out, 312, meta_t["d_inv"])
                p_acc = pool_merge("p", p_pool_out, 80, meta_t["p_inv"])

                def mlp2(accs, fchK, w1, b1, w2, b2, out_sb, orow):
                    z2p = hpsum.tile([64, G], F32, tag="hp2")
                    for s in range(8):
                        zp = hpsum.tile([P, G], F32, tag="hp")
                        for fi, (fo, fw) in enumerate(fchK):
                            nc.tensor.matmul(
                                out=zp[:], lhsT=w1[fi][:, s * P:(s + 1) * P],
                                rhs=accs[fi][:],
                                start=(fi == 0), stop=(fi == len(fchK) - 1))
                        zs = hpool.tile([P, G], F32, tag="z1")
                        nc.scalar.activation(zs[:], zp[:], AF.Relu,
                                             bias=b1[:, s:s + 1])
                        nc.tensor.matmul(out=z2p[:], lhsT=w2[s][:, :], rhs=zs[:],
                                         start=(s == 0), stop=(s == 7))
                    nc.scalar.activation(out_sb[orow:orow + 64, :], z2p[:],
                                         AF.Relu, bias=b2[:, 0:1])

                c_sb = hkeep.tile([P, G], F32, tag="c_sb")
                mlp2(d_acc, _fsplit(312), hw["dL1_w"], bsb["dL1_b"],
                     hw["dL2_w"], bsb["dL2_b"], c_sb, 0)
                mlp2(p_acc, _fsplit(80), hw["pL1_w"], bsb["pL1_b"],
                     hw["pL2_w"], bsb["pL2_b"], c_sb, 64)

                h1 = []
                for s in range(8):
                    hp = hpsum.tile([P, G], F32, tag="hp")
                    nc.tensor.matmul(out=hp[:],
                                     lhsT=hw["fW1"][0][:, s * P:(s + 1) * P],
                                     rhs=c_sb[:], start=True, stop=True)
                    hs = hkeep.tile([P, G], F32, tag=f"h1_{s}")
                    nc.scalar.activation(hs[:], hp[:], AF.Relu,
                                         bias=bsb["fb1"][:, s:s + 1])
                    h1.append(hs)
                h2 = []
                for s2 in range(4):
                    hp = hpsum.tile([P, G], F32, tag="hp2")
                    for s in range(8):
                        nc.tensor.matmul(
                            out=hp[:], lhsT=hw["fW2"][s][:, s2 * P:(s2 + 1) * P],
                            rhs=h1[s][:], start=(s == 0), stop=(s == 7))
                    hs = hkeep.tile([P, G], F32, tag=f"h2_{s2}")
                    nc.scalar.activation(hs[:], hp[:], AF.Relu,
                                         bias=bsb["fb2"][:, s2:s2 + 1])
                    h2.append(hs)
                op_ = hpsum.tile([1, G], F32, tag="hp")
                for s2 in range(4):
                    nc.tensor.matmul(out=op_[:], lhsT=hw["fW3"][s2][:, :],
                                     rhs=h2[s2][:], start=(s2 == 0),
                                     stop=(s2 == 3))
                ob = hpool.tile([1, G], F32, tag="ob")
                nc.vector.tensor_tensor(
                    out=ob[:], in0=op_[:],
                    in1=bsb["fb3"][:, 0:1].to_broadcast([1, G]), op=OP.add)
                nc.sync.dma_start(out_t[:], ob[:])
                if dbg:
                    def dump(dst, src2d, rows):
                        for r0 in range(0, rows, P):
                            rr = min(P, rows - r0)
                            t = hpool.tile([P, src2d.shape[1]], F32,
                                           tag="dbgt")
                            nc.sync.dma_start(t[:rr, :], src2d[r0:r0 + rr, :])
                            nc.sync.dma_start(dst[r0:r0 + rr, :], t[:rr, :])
                    dump(dbg["d_loc1"], d_loc1[:], SHARD_D)
                    dump(dbg["d_x3"], d_x3[:], SHARD_D)
                    dump(dbg["p_x3"], p_x3[:], SHARD_P)
                    dump(dbg["d_win"],
                         d_pool_in[:].rearrange("(f g) -> f g", g=P), 312)
                    dump(dbg["p_win"],
                         p_pool_in[:].rearrange("(f g) -> f g", g=P), 80)
                    nc.sync.dma_start(dbg["c_sb"][:], c_sb[:])

    nc.compile()
    return nc


# ------------------------------------------------------------------ kernel()
def kernel(**inputs):
    import time as _time
    _t0 = _time.perf_counter()
    cache = _CACHE

    md = _prep_tower(np.asarray(inputs["drug_edge_index"]),
                     np.asarray(inputs["drug_edge_attr"]), ND, SHARD_D)
    mp = _prep_tower(np.asarray(inputs["protein_edge_index"]),
                     np.asarray(inputs["protein_edge_attr"]), NP_, SHARD_P)
    d_pc, d_inv = _prep_pool(inputs["drug_batch"], ND, SHARD_D)
    p_pc, p_inv = _prep_pool(inputs["protein_batch"], NP_, SHARD_P)

    struct_key = (tuple(md["nchunks"]), tuple(mp["nchunks"]))
    if cache.get("struct_key") != struct_key:
        cache["nc"] = _build({"d": md, "p": mp})
        cache["struct_key"] = struct_key

    iota = np.tile(np.arange(P, dtype=np.float32)[None, :], (P, 1))

    def f32(a):
        return np.ascontiguousarray(np.asarray(a, np.float32))

    common = dict(
        drug_x=f32(inputs["drug_x"]), protein_x=f32(inputs["protein_x"]),
        iota_c=iota, d_inv=d_inv, p_inv=p_inv,
        dL1_b=f32(inputs["dL1_b"]).reshape(8, P).T.copy(),
        dL2_b=f32(inputs["dL2_b"]).reshape(64, 1),
        pL1_b=f32(inputs["pL1_b"]).reshape(8, P).T.copy(),
        pL2_b=f32(inputs["pL2_b"]).reshape(64, 1),
        fb1=f32(inputs["fb1"]).reshape(8, P).T.copy(),
        fb2=f32(inputs["fb2"]).reshape(4, P).T.copy(),
        fb3=f32(inputs["fb3"]).reshape(1, 1),
    )
    for n in ["dW1", "dW2", "dW3", "pW1", "pW2", "pW3", "dL1_w", "dL2_w",
              "pL1_w", "pL2_w", "fW1", "fW2", "fW3"]:
        common[n] = f32(inputs[n])

    in_maps = []
    for k in range(NCORE):
        im = dict(common)
        im.update(d_gidx=md["gidx"][k], d_nrm=md["nrm"][k], d_cl=md["cl"][k],
                  p_gidx=mp["gidx"][k], p_nrm=mp["nrm"][k], p_cl=mp["cl"][k],
                  d_pc=np.ascontiguousarray(d_pc[k]),
                  p_pc=np.ascontiguousarray(p_pc[k]))
        in_maps.append(im)

    cache["last_in_maps"] = in_maps
    cache["last_prep_s"] = _time.perf_counter() - _t0
    res = run_bass_kernel_spmd(cache["nc"], in_maps,
                               core_ids=list(range(NCORE)))
    cache["last_res"] = res.results
    return np.asarray(res.results[0]["out"], np.float32).reshape(G)

